# revision 22
# baseline (speedup 1.0000x reference)
"""VQ codebook (vector-quantization nearest-neighbor lookup) on Trainium2.

Problem: z [32,256,32,32] f32, codebook weight [1024,256] f32.
  flat = z transposed to channels-last, reshaped [32768, 256]
  dists[n,k] = ||flat_n||^2 - 2 flat_n . w_k + ||w_k||^2
  idx = argmin_k dists
  codes     = z_e + (q - z_e)   (elementwise, q = w[idx])
  codes_bar = q
  both returned in [B,C,H,W] layout.

Strategy (8 cores, data-parallel over batch; 4 batches/core):
  * scores[t,k] = flat_t . w_k - 0.5||w_k||^2  -> argmax_k == argmin_k dists.
    z[b] is already [C, HW] in DRAM = the natural lhsT layout (contraction
    dim C on partitions); rhs = host-precomputed w^T [C, K]; the -0.5||w||^2
    bias is folded in with a K=1 matmul (lhsT = ones row, rhs = bias row).
  * argmax via a one-pass custom DVE op (running-max scan + select + MAX
    accum of the index) reading scores straight out of PSUM.
  * gather q = w[idx] via indirect DMA ([128,8] u32 row indices per batch).
  * q arrives token-major [tok, C]; PE transposes (identity matmul) flip it
    to [C, tok]; codes = z + (qT - z) elementwise; both outputs DMA'd out.
"""

import sys

for _p in ("/opt/trn_rl_repo",):
    if _p not in sys.path:
        sys.path.insert(0, _p)

from contextlib import ExitStack

import numpy as np

import concourse.bass as bass
import concourse.mybir as mybir
import concourse.tile as tile
from concourse import bacc
from concourse.bass_utils import run_bass_kernel_spmd
from concourse.masks import make_identity

B, C, H, W = 32, 256, 32, 32
HW = H * W               # 1024 tokens per batch
K = 1024                 # codebook entries
NCORES = 8
BPC = B // NCORES        # batches per core
NTILE = HW // 128        # 128-token tiles per batch
F32 = mybir.dt.float32


# --------------------------------------------------------------------------
# custom DVE op: one-pass argmax along the free axis.
#   out[p,k]     = k if in0[p,k] == running_max(in0[p,:k+1]) else -1   (scratch)
#   accum_out[p] = max_k out[p,k]  == index of the max (last tie wins)
# --------------------------------------------------------------------------
_ARGMAX_NAME = "ARGMAX_BIAS_ANT"


def _register_argmax_op():
    """argmax of (Src0 + Src1) along the free axis, one pass.

    body[p,k]     = k if s[p,k] == running_max(s[p,:k+1]) else -1  (s = in0+in1)
    accum_out[p]  = max_k body[p,k]  == argmax index (last tie wins)

    in1 carries the -0.5*||w_k||^2 bias row broadcast to all partitions, so
    the PE matmul only computes x.w and the bias add rides along in the same
    DVE pass that does the argmax.
    """
    import concourse.dve_ops as dve_ops
    from concourse.dve_spec import (
        AluOp,
        Idx,
        One,
        Spec,
        Src0,
        Src1,
        Zero,
        eq,
        lower,
        scan,
        select,
        _has_src1,
    )
    from concourse.dve_uop import DveOpSpec

    for op in dve_ops.OPS:
        if op.name == _ARGMAX_NAME:
            return op

    def _ref(in0, in1, c0, c1, c2):
        x = np.asarray(in0, np.float32).astype(np.float32)
        x2 = x.reshape(x.shape[0], -1)
        if in1 is not None:
            y = np.asarray(in1, np.float32).reshape(x2.shape[0], -1)
            x2 = (x2 + y).astype(np.float32)
        r = np.maximum.accumulate(x2, axis=1)
        idx = np.arange(x2.shape[1], dtype=np.float32)
        body = np.where(x2 == r, idx, np.float32(-1.0)).astype(np.float32)
        acc = body.max(axis=1, keepdims=True)
        return body.reshape(x.shape), acc

    s = Src0 + Src1
    spec = Spec(
        body=select(eq(s, scan(AluOp.MAX, s)), Idx, Zero - One),
        accum=AluOp.MAX,
        reference=_ref,
    )

    row = max(dve_ops._SUB_OPCODE_FOR_NAME.values()) + 1
    dve_ops._SUB_OPCODE_FOR_NAME[_ARGMAX_NAME] = row

    shas = {}
    for ver in ("v3", "v4"):
        try:
            uops = lower(spec, ver=ver)
            shas[ver] = DveOpSpec(
                name=_ARGMAX_NAME, opcode=row, uops=uops, rd1_en=_has_src1(spec)
            ).sha(ver)
        except Exception:
            pass

    op = dve_ops.DveOp(
        name=_ARGMAX_NAME, spec=spec, subdim=False, uops_sha=shas
    )
    dve_ops.OPS.append(op)
    dve_ops.CUSTOM_DVE_SPECS[_ARGMAX_NAME] = spec
    return op


# --------------------------------------------------------------------------
# kernel builder
# --------------------------------------------------------------------------
def _build():
    argmax_op = _register_argmax_op()

    nc = bacc.Bacc(
        "TRN2", target_bir_lowering=False, debug=False, num_devices=NCORES
    )
    z_d = nc.dram_tensor("z", [BPC, C, HW], F32, kind="ExternalInput").ap()
    wT_d = nc.dram_tensor("wT", [C, K], F32, kind="ExternalInput").ap()
    b2_d = nc.dram_tensor("b2", [1, K], F32, kind="ExternalInput").ap()
    w_d = nc.dram_tensor("w", [K, C], F32, kind="ExternalInput").ap()
    codes_d = nc.dram_tensor(
        "codes", [BPC, C, HW], F32, kind="ExternalOutput"
    ).ap()
    cbar_d = nc.dram_tensor(
        "codes_bar", [BPC, C, HW], F32, kind="ExternalOutput"
    ).ap()

    with tile.TileContext(nc) as tc, ExitStack() as ctx:
        consts = ctx.enter_context(tc.tile_pool(name="consts", bufs=1))
        zp = ctx.enter_context(tc.tile_pool(name="zp", bufs=3))
        qp = ctx.enter_context(tc.tile_pool(name="qp", bufs=2))
        workp = ctx.enter_context(tc.tile_pool(name="workp", bufs=2))
        outp = ctx.enter_context(tc.tile_pool(name="outp", bufs=3))
        idxp = ctx.enter_context(tc.tile_pool(name="idxp", bufs=2))
        ps_s = ctx.enter_context(tc.tile_pool(name="ps_s", bufs=2, space="PSUM"))
        ps_q = ctx.enter_context(tc.tile_pool(name="ps_q", bufs=2, space="PSUM"))

        # constants — wT halves as separate tiles on the scalar HWDGE queue
        # (parallel with the z loads on the sync queue); the first matmuls
        # then only gate on wT_a.
        wT_r = wT_d.rearrange("(d p) k -> p d k", p=128)
        wT_a = consts.tile([128, K], F32, tag="wTa")
        nc.sync.dma_start(out=wT_a[:], in_=wT_r[:, 0, :])
        wT_b = consts.tile([128, K], F32, tag="wTb")
        nc.sync.dma_start(out=wT_b[:], in_=wT_r[:, 1, :])
        wT_sb = [wT_a[:], wT_b[:]]
        bias_sb = consts.tile([128, K], F32, tag="bias")
        b2_bcast = bass.AP(
            tensor=b2_d.tensor,
            offset=b2_d.offset,
            ap=[[0, 128]] + list(b2_d.ap[1:]),
        )
        nc.gpsimd.dma_start(out=bias_sb[:], in_=b2_bcast)
        ident = consts.tile([128, 128], F32, tag="ident")
        make_identity(nc, ident[:])

        def dist_phase(b):
            """Distance matmuls + argmax + gather for batch b.
            Returns (z_sb, q_sb) needed by the output phase."""
            z2 = zp.tile([128, 2, HW], F32, tag="z")
            z_r = z_d[b].rearrange("(d p) hw -> p d hw", p=128)
            z_first = None
            nc.sync.dma_start(out=z2[:], in_=z_r)
            z_sb = [z2[:, 0, :], z2[:, 1, :]]

            idxf = idxp.tile([128, NTILE], F32, tag="idxf")
            idxu = idxp.tile([128, NTILE], mybir.dt.uint32, tag="idxu")
            q_sb = qp.tile([128, NTILE, C], F32, tag="q")
            for j in range(NTILE):
                ps = ps_s.tile([128, K], F32, space="PSUM")
                tok = slice(j * 128, (j + 1) * 128)
                for d in range(2):
                    if z_first is not None and j == 0:
                        lhsT = z_first[:, d, :]
                    else:
                        lhsT = z_sb[d][:, tok]
                    for kb in range(2):
                        sl = slice(kb * 512, (kb + 1) * 512)
                        nc.tensor.matmul(
                            ps[:, sl], lhsT=lhsT, rhs=wT_sb[d][:, sl],
                            start=(d == 0), stop=(d == 1),
                        )
                trash = workp.tile([128, K], F32, tag="trash")
                nc.vector._custom_dve(
                    argmax_op,
                    out=trash[:],
                    in0=ps[:, :],
                    in1=bias_sb[:],
                    accum_out=idxf[:, j : j + 1],
                )
                # per-tile cast + gather: the gather (and the transposes
                # behind it) can start while later tiles are still in the
                # matmul stream.
                nc.vector.tensor_copy(
                    out=idxu[:, j : j + 1], in_=idxf[:, j : j + 1]
                )
                nc.gpsimd.indirect_dma_start(
                    out=q_sb[:, j, :],
                    out_offset=None,
                    in_=w_d[:],
                    in_offset=bass.IndirectOffsetOnAxis(
                        ap=idxu[:, j : j + 1], axis=0
                    ),
                )
            return z_sb, q_sb

        def out_phase(b, z_sb, q_sb):
            """PE-transpose q, then codes/codes_bar writes for batch b."""
            for cb in range(2):
                qt = ps_q.tile([128, HW], F32, space="PSUM")
                for j in range(NTILE):
                    nc.tensor.transpose(
                        out=qt[:, j * 128 : (j + 1) * 128],
                        in_=q_sb[:, j, cb * 128 : (cb + 1) * 128],
                        identity=ident[:],
                    )
                crow = slice(cb * 128, (cb + 1) * 128)
                cbar_sb = outp.tile([128, HW], F32, tag="cbar")
                nc.scalar.copy(out=cbar_sb[:], in_=qt[:])
                nc.sync.dma_start(out=cbar_d[b, crow, :], in_=cbar_sb[:])

                diff = workp.tile([128, HW], F32, tag="diff")
                nc.vector.tensor_sub(out=diff[:], in0=qt[:], in1=z_sb[cb][:])
                codes_sb = outp.tile([128, HW], F32, tag="codes")
                nc.vector.tensor_add(
                    out=codes_sb[:], in0=z_sb[cb][:], in1=diff[:]
                )
                nc.sync.dma_start(out=codes_d[b, crow, :], in_=codes_sb[:])

        # Software pipeline: batch b's output phase is emitted after batch
        # b+1's distance phase, so the PE transposes never stall on the
        # gather DMA and the matmul stream stays dense (HAM stays warm).
        prev = None
        for b in range(BPC):
            cur = (b, *dist_phase(b))
            if prev is not None:
                out_phase(*prev)
            prev = cur
        out_phase(*prev)

    nc.compile()
    return nc


def _maybe_enable_ldw_opt():
    """Walrus elides redundant LDWEIGHTS when --enable-ldw-opt=true; bass
    hardcodes false. Consecutive distance matmuls here share the same
    stationary z-tile, so this saves a reload per pair. Gated for A/B."""
    import os

    from concourse import bass_utils as _bu

    if not os.environ.get("BASS_LDW_OPT"):
        return
    if getattr(_bu, "_ldw_patched", False):
        return
    orig = _bu.run_command

    def patched(argv, **kw):
        argv = [
            "--enable-ldw-opt=true" if a == "--enable-ldw-opt=false" else a
            for a in argv
        ]
        return orig(argv, **kw)

    _bu.run_command = patched
    _bu._ldw_patched = True


_CACHE = {}


def _get_nc():
    if "nc" not in _CACHE:
        _CACHE["nc"] = _build()
    return _CACHE["nc"]


def _setup_profile_hook():
    """Install the NTFF profiling hook when the image lacks antenv.axon_hooks
    (the boot shim degrades silently without it), and disable the artifact
    upload (no egress here)."""
    import types

    from concourse import bass_utils as _bu

    _bu.upload_artifacts = lambda tmpdir: tmpdir
    try:
        import antenv.axon_hooks  # noqa: F401

        return
    except ImportError:
        pass
    import antenv

    mod = types.ModuleType("antenv.axon_hooks")
    _box = [None]
    mod.set_axon_ntff_profile_hook = lambda h: _box.__setitem__(0, h)
    mod.get_axon_ntff_profile_hook = lambda: _box[0]
    sys.modules["antenv.axon_hooks"] = mod
    antenv.axon_hooks = mod
    try:
        from trn_agent_boot.trn_boot import _ntff_profile_via_ctypes

        hook = _ntff_profile_via_ctypes("/opt/axon/libaxon_pjrt.so")
        if hook is not None:
            mod.set_axon_ntff_profile_hook(hook)
    except Exception:
        pass


def _run(z, weight, trace=False, tmpdir=None):
    z = np.ascontiguousarray(np.asarray(z, dtype=np.float32))
    w = np.ascontiguousarray(np.asarray(weight, dtype=np.float32))
    assert z.shape == (B, C, H, W), z.shape
    assert w.shape == (K, C), w.shape

    wT = np.ascontiguousarray(w.T)
    b2 = (-0.5 * (w.astype(np.float64) ** 2).sum(axis=1)).astype(np.float32)
    b2 = np.ascontiguousarray(b2[None, :])

    z3 = z.reshape(B, C, HW)
    in_maps = []
    for i in range(NCORES):
        in_maps.append(
            {
                "z": np.ascontiguousarray(z3[i * BPC : (i + 1) * BPC]),
                "wT": wT,
                "b2": b2,
                "w": w,
            }
        )

    _maybe_enable_ldw_opt()
    if trace:
        _setup_profile_hook()
    res = run_bass_kernel_spmd(
        _get_nc(),
        in_maps,
        core_ids=list(range(NCORES)),
        trace=trace,
        tmpdir=tmpdir,
    )
    codes = np.concatenate([r["codes"] for r in res.results], axis=0)
    cbar = np.concatenate([r["codes_bar"] for r in res.results], axis=0)
    codes = codes.reshape(B, C, H, W)
    cbar = cbar.reshape(B, C, H, W)
    return (codes, cbar), res


def kernel(z, weight):
    (codes, cbar), _ = _run(z, weight, trace=False)
    return codes, cbar


def kernel_timed(z, weight):
    (codes, cbar), res = _run(z, weight, trace=True)
    return (codes, cbar), res
